# revision 31
# baseline (speedup 1.0000x reference)
"""BotRGCN Trainium2 kernel v3, 8-way SPMD.

Key changes vs v2:
- Relation-major scatter columns (ct = r*128 + dstrow): windows pack densely,
  fewer slots, contiguous per-relation epilogue slices.
- CHMAX=16 gather chunks (amortize ~1us SWDGE fixed overhead per call).
- h AllGather split into 4 section tables, issued inside the MLP loop.
- h1 AllGather split into 3 section tables, issued inside the L1 tile loop.
- Deferred last-section processing (D tiles) hides the final AllGather chunk.
- MLP processes 4 tiles per PSUM group (N=512) with text DMA on 4 engines.
- A matrices streamed in emission order (single sequential stream).
"""
import os
import sys

for _p in ("/opt/trn_rl_repo", "/root/.axon_site/_ro/trn_rl_repo"):
    if os.path.isdir(_p) and _p not in sys.path:
        sys.path.insert(0, _p)

import numpy as np
import ml_dtypes

from concourse import bass, bacc, tile, mybir
from concourse.bass_utils import run_bass_kernel_spmd

BF16 = ml_dtypes.bfloat16

N_NODES = 50000
N_REL = 3
FEAT = 128
VAL = 16
TEXT = 768
CLASSES = 2
CORES = 8
P = 128
W = 128
CHMAX = 8
ABATCH = 16
AGEN = False
RSLOT = 4
NSP = ((N_NODES // CORES) + P - 1) // P * P  # 6272
NT_MLP = NSP // P                            # 49
TC = TEXT // P                               # 6

# h table sections: MLP tile boundaries (per shard)
SB1 = [0, 16, 32, 49]
NSEC1 = len(SB1) - 1
# h1 table sections: L1 tile boundaries (set in make_plan from nt1)
NSEC2 = 2
D1 = 5   # L1 deferred tiles for last section
D2 = 3   # L2 deferred tiles for last section
AG_L2_AT_END = True  # issue all h1 AllGathers after L1 completes


def wrap16(flat):
    L = len(flat)
    assert L % 16 == 0
    a = np.asarray(flat, np.int16).reshape(-1, 16).T
    return np.ascontiguousarray(np.tile(a, (8, 1)))


# ============================ planner ================================


class Plan:
    pass


def _build_schedule(cts, cmax, w=W, cap=P):
    """Joint (cross-core) slot schedule for one (tile, section).
    cts: list of sorted int arrays (ct keys in [0, cmax)).
    Returns (bases, ranges)."""
    n = len(cts)
    ptrs = [0] * n
    lens = [len(a) for a in cts]
    bases, ranges = [], [[] for _ in range(n)]
    while any(ptrs[c] < lens[c] for c in range(n)):
        b = min(cts[c][ptrs[c]] for c in range(n) if ptrs[c] < lens[c])
        b = min(int(b), cmax - w)
        bases.append(b)
        for c in range(n):
            s = ptrs[c]
            hi = int(np.searchsorted(cts[c], b + w, side="left"))
            e = min(s + cap, max(hi, s))
            ranges[c].append((s, e))
            ptrs[c] = e
    return bases, ranges


def make_sigma(edge_index, edge_type, idx, n_nodes, cores, nsp):
    """Node permutation + pruning sets."""
    src = np.asarray(edge_index[0], np.int64)
    dst = np.asarray(edge_index[1], np.int64)

    idxset = np.unique(np.asarray(idx, np.int64))
    in_idx = np.zeros(n_nodes, bool)
    in_idx[idxset] = True

    m2 = in_idx[dst]                      # L2 edges
    l2src = np.unique(src[m2])
    needed = np.zeros(n_nodes, bool)
    needed[l2src] = True
    needed[idxset] = True
    m1 = needed[dst]                      # L1 edges

    D = idxset                             # idx dsts
    O = np.setdiff1d(np.flatnonzero(needed), D, assume_unique=False)
    U = np.flatnonzero(~needed)

    sd = [D[c::cores] for c in range(cores)]
    so = [O[c::cores] for c in range(cores)]
    su = [U[c::cores] for c in range(cores)]
    nd = max(len(x) for x in sd)
    nneed = max(len(sd[c]) + len(so[c]) for c in range(cores))
    nt2 = (nd + P - 1) // P
    nt1 = (nneed + P - 1) // P
    shard_nodes = np.full((cores, nsp), -1, np.int64)
    pos = np.full(n_nodes, -1, np.int64)
    for c in range(cores):
        arr = np.concatenate([sd[c], so[c], su[c]])
        assert len(arr) <= nsp, (len(arr), nsp)
        shard_nodes[c, :len(arr)] = arr
        pos[arr] = c * nsp + np.arange(len(arr))
    return dict(pos=pos, shard_nodes=shard_nodes, nt1=nt1, nt2=nt2,
                m1=m1, m2=m2, in_idx=in_idx, needed=needed,
                nd_per_core=np.array([len(x) for x in sd]))


def emission_events(lp, D):
    """Order in which the builder consumes slots.
    Yields ('open', t), ('slot', s, j), ('close', t).
    Sections 0..nsec-2 of tile t are emitted at t; the last section and the
    tile epilogue are deferred by D tiles."""
    ev = []
    nt, last = lp.nt, lp.nsec - 1
    for t in range(nt + D):
        if t < nt:
            ev.append(("open", t))
            for s in range(last):
                a, b = lp.tile_slot_range[s][t]
                ev += [("slot", s, j) for j in range(a, b)]
        if t >= D:
            td = t - D
            a, b = lp.tile_slot_range[last][td]
            ev += [("slot", last, j) for j in range(a, b)]
            ev.append(("close", td))
    return ev


def layer_plan(erow, ect, eowner, etile, enorm, cores, nt, sec, nsec, D):
    """Build joint slot schedule for one layer.

    erow: table-relative row per entry (int16 range); ect: ct key;
    eowner: dst core; etile: dst tile; enorm: edge norm; sec: table
    selector per entry in [0, nsec)."""
    SECS = tuple(range(nsec))
    order = np.lexsort((ect, sec, etile, eowner))
    erow, ect, sec = erow[order], ect[order], sec[order]
    eowner, etile, enorm = eowner[order], etile[order], enorm[order]

    key = (eowner * nt + etile) * nsec + sec
    bounds = np.searchsorted(key, np.arange(cores * nt * nsec + 1))

    slot_base = {s: [] for s in SECS}
    tile_slot_range = {s: np.zeros((nt, 2), np.int64) for s in SECS}
    idx16 = {s: [[] for _ in range(cores)] for s in SECS}
    acols = {s: [[] for _ in range(cores)] for s in SECS}
    anrm = {s: [[] for _ in range(cores)] for s in SECS}

    for t in range(nt):
        for s in SECS:
            cts_, rows_, nrms_ = [], [], []
            for c in range(cores):
                k = (c * nt + t) * nsec + s
                a, b = bounds[k], bounds[k + 1]
                cts_.append(ect[a:b])
                rows_.append(erow[a:b])
                nrms_.append(enorm[a:b])
            start = len(slot_base[s])
            bases, ranges = _build_schedule(cts_, P * RSLOT)
            for bj in bases:
                slot_base[s].append(bj)
            for c in range(cores):
                for j, (a, b) in enumerate(ranges[c]):
                    n = b - a
                    r = rows_[c][a:b]
                    cc = cts_[c][a:b] - bases[j]
                    nn = nrms_[c][a:b]
                    so_ = np.argsort(r, kind="stable")
                    r, cc, nn = r[so_], cc[so_], nn[so_]
                    ii = np.zeros(P, np.int16)
                    col = np.full(P, -1, np.int64)
                    nrm = np.zeros(P, np.float32)
                    assert n == 0 or r.max() < 32768
                    ii[:n] = r.astype(np.int16)
                    col[:n] = cc
                    nrm[:n] = nn
                    idx16[s][c].append(ii)
                    acols[s][c].append(col)
                    anrm[s][c].append(nrm)
            tile_slot_range[s][t] = (start, len(slot_base[s]))

    ns = [len(slot_base[s]) for s in SECS]
    nslot = sum(ns)
    out = Plan()
    out.nt, out.nslot, out.ns, out.nsec, out.D = nt, nslot, ns, nsec, D
    out.slot_base = {s: np.array(slot_base[s], np.int64) for s in SECS}
    out.tile_slot_range = tile_slot_range

    # per-core packed: idx streams per sec; A matrix in EMISSION order
    emit = emission_events(out, D)
    out.emit = emit
    slots_in_order = [(s, j) for (k, s, *r) in
                      [(e[0], e[1], *e[2:]) for e in emit] if False]
    slots_in_order = [(e[1], e[2]) for e in emit if e[0] == "slot"]
    out.slots_in_order = slots_in_order
    assert len(slots_in_order) == nslot
    out.idx = {}
    out.cn = {}
    out.amat = {}
    for c in range(cores):
        out.idx[c] = [
            (np.stack(idx16[s][c]) if idx16[s][c] else np.zeros((0, P), np.int16))
            for s in SECS
        ]
        colv = np.full((P, max(nslot, 1)), -1.0, np.float32)
        nrmv = np.zeros((P, max(nslot, 1)), np.float32)
        for g, (s, j) in enumerate(slots_in_order):
            colv[:, g] = acols[s][c][j].astype(np.float32)  # -1 for pads
            nrmv[:, g] = anrm[s][c][j]
        out.cn[c] = (colv, nrmv)
        am = np.zeros((P, max(nslot, 1) * W), BF16)
        for g, (s, j) in enumerate(slots_in_order):
            col = acols[s][c][j]
            nrm = anrm[s][c][j]
            v = col >= 0
            am[np.flatnonzero(v), g * W + col[v]] = nrm[v].astype(BF16)
        out.amat[c] = am
    return out


def make_plan(edge_index, edge_type, idx, n_nodes=50000, cores=8):
    src = np.asarray(edge_index[0], np.int64)
    dst = np.asarray(edge_index[1], np.int64)
    et = np.asarray(edge_type, np.int64)

    nsp = ((n_nodes // cores) + P - 1) // P * P  # 6272
    sg = make_sigma(edge_index, edge_type, idx, n_nodes, cores, nsp)
    pos, nt1, nt2 = sg["pos"], sg["nt1"], sg["nt2"]

    deg = np.zeros((N_REL, n_nodes), np.int64)
    np.add.at(deg, (et, dst), 1)
    norm = 1.0 / np.maximum(deg[et, dst], 1).astype(np.float32)

    pl = Plan()
    pl.cores, pl.nsp = cores, nsp
    pl.nt_mlp = nsp // P
    pl.sigma = sg
    pl.nt1, pl.nt2 = nt1, nt2

    # ---- L1: h split into NSEC1 section tables by MLP tile ranges
    rb1 = [b * P for b in SB1]              # per-shard row bases
    pl.rb1 = rb1
    rows1 = [rb1[s + 1] - rb1[s] for s in range(NSEC1)]
    pl.rows1 = rows1
    m1 = sg["m1"]
    s1, d1, r1, n1 = src[m1], dst[m1], et[m1], norm[m1]
    dpos = pos[d1]
    owner = dpos // nsp
    loc = dpos % nsp
    spos = pos[s1]
    sc, sl = spos // nsp, spos % nsp
    sec = np.searchsorted(np.array(rb1[1:]), sl, side="right")
    base = np.array([rb1[s] for s in range(NSEC1)])[sec]
    rows_s = np.array(rows1)[sec]
    erow = sc * rows_s + (sl - base)
    assert all(erow[sec == s].max(initial=0) < 32768 for s in range(NSEC1))
    ect = r1 * P + (loc % P)                # relation-major
    etile = loc // P
    assert (etile < nt1).all()
    pl.L1 = layer_plan(erow, ect, owner, etile, n1, cores, nt1, sec,
                       nsec=NSEC1, D=D1)

    # ---- L2: h1 split into NSEC2 section tables by L1 tile ranges
    tb2 = [0, min(32, nt1 - 1), nt1] if NSEC2 == 2 else \
        [min(k * ((nt1 + NSEC2 - 1) // NSEC2), nt1) for k in range(NSEC2 + 1)]
    pl.tb2 = tb2
    rb2 = [b * P for b in tb2]
    rows2 = [rb2[s + 1] - rb2[s] for s in range(NSEC2)]
    pl.rows2 = rows2
    m2 = sg["m2"]
    s2, d2, r2, n2 = src[m2], dst[m2], et[m2], norm[m2]
    dpos = pos[d2]
    owner = dpos // nsp
    loc = dpos % nsp
    assert (loc < nt2 * P).all()
    spos = pos[s2]
    sc, sl = spos // nsp, spos % nsp
    assert (sl < nt1 * P).all()
    sec = np.searchsorted(np.array(rb2[1:]), sl, side="right")
    base = np.array(rb2)[sec]
    rows_s = np.array(rows2)[sec]
    erow = sc * rows_s + (sl - base)
    assert all(erow[sec == s].max(initial=0) < 32768 for s in range(NSEC2))
    ect = r2 * P + (loc % P)
    etile = loc // P
    pl.L2 = layer_plan(erow, ect, owner, etile, n2, cores, nt2, sec,
                       nsec=NSEC2, D=D2)
    return pl


# ============================ blob layout =============================

def blob_layout(pl):
    n1, n2 = pl.L1.nslot, pl.L2.nslot
    segs = [
        ("textT", [NT_MLP, P, TC * P]),
        ("valT", [VAL, NSP]),
        ("fc1w", [VAL, FEAT]),
        ("fc2w", [P, TC * P]),
        ("rwv", [FEAT, FEAT]),
        ("rwt", [FEAT, FEAT]),
        ("beff", [1, FEAT]),
        ("ww1", [P, RSLOT * FEAT]),
        ("b1", [1, FEAT]),
        ("ww2", [P, RSLOT * FEAT]),
        ("b2", [1, FEAT]),
        ("fc3w", [FEAT, CLASSES]),
        ("fc3b", [1, CLASSES]),
        ("ones1", [1, P]),
        ("selfA", [P, RSLOT * P]),
        ("a1", [P, max(n1, 1) * W]),
        ("a2", [P, max(n2, 1) * W]),
        ("col1", [P, max(n1, 1) * 2]),
        ("nrm1", [P, max(n1, 1) * 2]),
        ("col2", [P, max(n2, 1) * 2]),
        ("nrm2", [P, max(n2, 1) * 2]),
    ] + [
        (f"idx1s{s}", [P, max(pl.L1.ns[s], 1) * 8]) for s in range(NSEC1)
    ] + [
        (f"idx2s{s}", [P, max(pl.L2.ns[s], 1) * 8]) for s in range(NSEC2)
    ]
    out = {}
    off = 0
    for name, shape in segs:
        n = int(np.prod(shape))
        out[name] = (off, n, shape)
        off += ((n + 127) // 128) * 128
    return out, off


# ============================ bass builder =============================

def build_bass(pl, ablate=()):
    ab = set(ablate)
    NT1, NT2 = pl.nt1, pl.nt2

    nc = bacc.Bacc("TRN2", target_bir_lowering=False, debug=False,
                   num_devices=CORES, num_swdge_queues=4,
                   dynamic_dma_scratch_size=49152)
    qrr = {"n": 0}
    dt = mybir.dt
    f32, bf, i16 = dt.float32, dt.bfloat16, dt.int16

    layout, blob_n = blob_layout(pl)
    p_blob = nc.declare_dram_parameter("blob", [1, blob_n], bf, isOutput=False)
    p_logT = nc.declare_dram_parameter("logitsT", [CLASSES, NT2 * P], f32,
                                       isOutput=True)

    def seg(name, dtype=bf):
        off, n, shape = layout[name]
        ap = p_blob[0:1, off:off + n]
        if dtype != bf:
            ap = ap.bitcast(dtype)
        r = int(np.prod(shape[:-1]))
        return ap.rearrange("o (r c) -> (o r) c", r=r)

    with tile.TileContext(nc) as tc:
        with tc.tile_pool(name="wt", bufs=1) as wt, \
             tc.tile_pool(name="sb", bufs=2) as sb, \
             tc.tile_pool(name="ep", bufs=12) as ep, \
             tc.tile_pool(name="ab1", bufs=6) as abp, \
             tc.tile_pool(name="tts", bufs=3) as tts, \
             tc.tile_pool(name="dram", bufs=1, space="DRAM") as dram:

            def resident(name, dtype=bf):
                off, n, shape = layout[name]
                t = wt.tile(list(shape[-2:] if len(shape) == 2 else shape),
                            dtype, tag=name)
                nc.sync.dma_start(t[:], seg(name, dtype))
                return t

            fc1w = resident("fc1w")
            valT = resident("valT")
            fc2w = resident("fc2w")
            rwv = resident("rwv")
            rwt = resident("rwt")
            beff = resident("beff")
            ones1 = resident("ones1")

            hall1 = wt.tile([P, NT1, P], bf, tag="hall1")
            hall2 = wt.tile([P, NT2, P], bf, tag="hall2")

            h_shard = dram.tile([NSP, FEAT], bf)
            _as = "Shared" if "coll" not in ab else "Local"
            h_full = [dram.tile([CORES * pl.rows1[s], FEAT], bf,
                                addr_space=_as, name=f"h_full{s}")
                      for s in range(NSEC1)]
            h1_shard = dram.tile([NT1 * P, FEAT], bf)
            h1_full = [dram.tile([CORES * pl.rows2[s], FEAT], bf,
                                 addr_space=_as, name=f"h1_full{s}")
                       for s in range(NSEC2)]
            warm_in = dram.tile([P, 16], bf)
            warm_out = dram.tile([CORES * P, 16], bf, addr_space=_as)

            def allgather(src_ap, dst):
                nc.gpsimd.collective_compute(
                    "AllGather", mybir.AluOpType.bypass,
                    replica_groups=[list(range(CORES))],
                    ins=[src_ap], outs=[dst.opt()])

            selfA = resident("selfA")
            iot16 = wt.tile([P, W], mybir.dt.int16, tag="iot16")
            nc.gpsimd.iota(iot16[:], pattern=[[1, W]], base=0,
                           channel_multiplier=0)
            iotb = wt.tile([P, W], bf, tag="iotb")
            nc.vector.tensor_copy(out=iotb[:], in_=iot16[:])

            def resident_f32(name, cols):
                t = wt.tile([P, cols], f32, tag=name)
                nc.sync.dma_start(t[:], seg(name, f32))
                return t

            cnr = {
                1: (resident_f32("col1", max(pl.L1.nslot, 1)),
                    resident_f32("nrm1", max(pl.L1.nslot, 1))),
                2: (resident_f32("col2", max(pl.L2.nslot, 1)),
                    resident_f32("nrm2", max(pl.L2.nslot, 1))),
            }
            ww1 = resident("ww1")
            b1 = resident("b1")
            ww2 = resident("ww2")
            b2 = resident("b2")
            fc3w = resident("fc3w")
            fc3b = resident("fc3b")
            idxsb = {
                1: [resident(f"idx1s{s}", i16) for s in range(NSEC1)],
                2: [resident(f"idx2s{s}", i16) for s in range(NSEC2)],
            }


            # ================= phase 1: feature MLP (quad tiles) ======
            text_engines = [nc.sync, nc.scalar, nc.sync]
            with tc.tile_pool(name="ps1", bufs=2, space="PSUM") as ps1:
                for t0 in range(0, NT_MLP, 4):
                    quad = min(4, NT_MLP - t0)
                    tt = tts.tile([P, TC, 4, P], bf, tag="tt")
                    for h_ in range(quad):
                        toff = layout["textT"][0] + (t0 + h_) * P * TC * P
                        text_engines[(t0 + h_) % 2].dma_start(
                            tt[:, :, h_, :],
                            p_blob[0:1, toff:toff + P * TC * P]
                            .rearrange("o (p c n) -> (o p) c n", p=P, c=TC))
                    np_ = quad * P
                    pvT = ps1.tile([P, 4, P], f32, tag="pvT", space="PSUM")
                    nc.tensor.matmul(out=pvT[:, 0:quad, :], lhsT=fc1w[:],
                                     rhs=valT[:, t0 * P:t0 * P + np_],
                                     start=True, stop=True)
                    vT = sb.tile([P, 4, P], bf, tag="vT")
                    nc.vector.tensor_copy(out=vT[:, 0:quad, :],
                                          in_=pvT[:, 0:quad, :])
                    ptT = ps1.tile([P, 4, P], f32, tag="ptT", space="PSUM")
                    for c in range(TC):
                        nc.tensor.matmul(out=ptT[:, 0:quad, :],
                                         lhsT=fc2w[:, c * P:(c + 1) * P],
                                         rhs=tt[:, c, 0:quad, :],
                                         start=(c == 0), stop=(c == TC - 1))
                    tT = sb.tile([P, 4, P], bf, tag="tT")
                    nc.vector.tensor_copy(out=tT[:, 0:quad, :],
                                          in_=ptT[:, 0:quad, :])
                    ph = ps1.tile([P, 4, P], f32, tag="ph", space="PSUM")
                    for h_ in range(quad):
                        nc.tensor.matmul(out=ph[:, h_, :], lhsT=vT[:, h_, :],
                                         rhs=rwv[:], start=True, stop=False)
                        nc.tensor.matmul(out=ph[:, h_, :], lhsT=tT[:, h_, :],
                                         rhs=rwt[:], start=False, stop=False)
                        nc.tensor.matmul(out=ph[:, h_, :], lhsT=ones1[:],
                                         rhs=beff[:], start=False, stop=True)
                    for h_ in range(quad):
                        t = t0 + h_
                        if t < NT1:
                            hdst = hall1[:, t, :]
                        else:
                            hsb = sb.tile([P, P], bf, tag="hsb")
                            hdst = hsb[:]
                        nc.scalar.activation(
                            out=hdst, in_=ph[:, h_, :],
                            func=mybir.ActivationFunctionType.Lrelu,
                            alpha=0.01)
                        nc.sync.dma_start(
                            h_shard[t * P:(t + 1) * P, :], hdst)
                        if (t + 1) in SB1[1:] and "coll" not in ab:
                            k = SB1.index(t + 1) - 1
                            allgather(
                                h_shard[pl.rb1[k]:pl.rb1[k + 1], :],
                                h_full[k])
            if "coll" in ab:
                for k in range(NSEC1):
                    nc.sync.dma_start(h_full[k][0:pl.rows1[k], :],
                                      h_shard[pl.rb1[k]:pl.rb1[k + 1], :])

            # ================= RGCN layers =================
            A_ENGINES = [nc.scalar, nc.sync]
            def rgcn_layer(lp, src_tabs, hall, ww, bb, layer, out_shards,
                           li, ag_after=None):
                nsec = lp.nsec
                emitted = {s: -1 for s in range(nsec)}
                ebufs = {s: {} for s in range(nsec)}
                nstream = {s: lp.ns[s] for s in range(nsec)}
                colr, nrmr = cnr[li]
                aemitted = [-1]
                abufs = {}
                aseg = seg("a1" if li == 1 else "a2")
                # global emission index per (s, j)
                gidx = {sj: g for g, sj in enumerate(lp.slots_in_order)}

                def emit_chunk(s, ci):
                    s0 = ci * CHMAX
                    ns = min(CHMAX, nstream[s] - s0)
                    et = ep.tile([P, CHMAX, FEAT], bf, tag=f"e{s}")
                    if "gather" in ab:
                        nc.vector.memset(et[:, 0:1, 0:2], 0.0)
                        ebufs[s][ci] = (et, s0, ns)
                        ebufs[s].pop(ci - 12, None)
                        return
                    qrr["n"] += 1
                    qn = qrr["n"] % 4
                    nc.gpsimd.dma_gather(
                        out_ap=et[:, 0:ns, :],
                        in_ap=src_tabs[s],
                        idxs_ap=idxsb[li][s][:, s0 * 8:(s0 + ns) * 8],
                        num_idxs=ns * P,
                        num_idxs_reg=ns * P,
                        elem_size=FEAT,
                        queue_num=qn)
                    ebufs[s][ci] = (et, s0, ns)
                    ebufs[s].pop(ci - 12, None)

                def gen_a(g):
                    at = abp.tile([P, W], bf, tag="ag", name="at")
                    nc.vector.tensor_scalar(
                        out=at[:], in0=iotb[:],
                        scalar1=colr[:, g:g + 1],
                        scalar2=nrmr[:, g:g + 1],
                        op0=mybir.AluOpType.is_equal,
                        op1=mybir.AluOpType.mult)
                    return at

                def emit_abatch(ai):
                    s0 = ai * ABATCH
                    ns = min(ABATCH, lp.nslot - s0)
                    at = abp.tile([P, ABATCH, W], bf, tag="a", name="abt")
                    A_ENGINES[ai % len(A_ENGINES)].dma_start(
                        at[:, 0:ns, :],
                        aseg[:, s0 * W:(s0 + ns) * W])
                    abufs[ai] = (at, s0, ns)
                    abufs.pop(ai - 4, None)

                with tc.tile_pool(name=f"psl{li}", bufs=lp.D + 1,
                                  space="PSUM") as psl, \
                     tc.tile_pool(name=f"pso{li}", bufs=2,
                                  space="PSUM") as pso:
                    pS = {}
                    for e in lp.emit:
                        if e[0] == "open":
                            t = e[1]
                            pS[t] = psl.tile([P, RSLOT * P], f32, tag="pS",
                                             name="pS", space="PSUM")
                            # root one-hot: also initializes all 512 cols
                            nc.tensor.matmul(out=pS[t][:], lhsT=hall[:, t, :],
                                             rhs=selfA[:], start=True,
                                             stop=False, skip_group_check=True)
                        elif e[0] == "slot":
                            s, j = e[1], e[2]
                            t = None  # tile known via pS accumulation target
                            ci = j // CHMAX
                            g = gidx[(s, j)]
                            if ci > emitted[s]:
                                emit_chunk(s, ci)
                                emitted[s] = ci
                            et, es0, _ = ebufs[s][ci]
                            if AGEN:
                                at = gen_a(g)
                                a_ap = at[:]
                            else:
                                ai = g // ABATCH
                                if ai > aemitted[0]:
                                    emit_abatch(ai)
                                    aemitted[0] = ai
                                at, as0, _ = abufs[ai]
                                a_ap = at[:, g - as0, :]
                            bj = int(lp.slot_base[s][j])
                            if "slotmm" in ab:
                                continue
                            tt_ = e[3]
                            nc.tensor.matmul(
                                out=pS[tt_][:, bj:bj + W],
                                lhsT=et[:, j - es0, :],
                                rhs=a_ap,
                                start=False, stop=False,
                                skip_group_check=True)
                        else:  # close
                            t = e[1]
                            sS = sb.tile([P, RSLOT * P], bf, tag="sS")
                            nc.scalar.activation(
                                out=sS[:], in_=pS[t][:],
                                func=mybir.ActivationFunctionType.Copy)
                            del pS[t]
                            if layer == 1:
                                pO = pso.tile([P, FEAT], f32, tag="pO",
                                              space="PSUM")
                                for r in range(RSLOT):
                                    nc.tensor.matmul(
                                        out=pO[:],
                                        lhsT=sS[:, r * P:(r + 1) * P],
                                        rhs=ww[:, r * FEAT:(r + 1) * FEAT],
                                        start=(r == 0), stop=False)
                                nc.tensor.matmul(out=pO[:], lhsT=ones1[:],
                                                 rhs=bb[:], start=False,
                                                 stop=True)
                                if t < NT2:
                                    hodst = hall2[:, t, :]
                                else:
                                    hot = sb.tile([P, P], bf, tag="ho")
                                    hodst = hot[:]
                                nc.vector.tensor_copy(out=hodst, in_=pO[:])
                                nc.sync.dma_start(
                                    out_shards[t * P:(t + 1) * P, :], hodst)
                            else:
                                pO = pso.tile([P, P], f32, tag="pO",
                                              space="PSUM")
                                for r in range(RSLOT):
                                    nc.tensor.matmul(
                                        out=pO[:],
                                        lhsT=ww[:, r * FEAT:(r + 1) * FEAT],
                                        rhs=sS[:, r * P:(r + 1) * P],
                                        start=(r == 0), stop=False)
                                nc.tensor.matmul(out=pO[:], lhsT=bb[:],
                                                 rhs=ones1[:], start=False,
                                                 stop=True)
                                h2T = sb.tile([P, P], bf, tag="h2T")
                                nc.vector.tensor_copy(out=h2T[:], in_=pO[:])
                                pL = pso.tile([CLASSES, P], f32, tag="pL",
                                              space="PSUM")
                                nc.tensor.matmul(out=pL[:], lhsT=fc3w[:],
                                                 rhs=h2T[:], start=True,
                                                 stop=False)
                                nc.tensor.matmul(out=pL[:], lhsT=fc3b[:],
                                                 rhs=ones1[:], start=False,
                                                 stop=True)
                                lg = sb.tile([CLASSES, P], f32, tag="lg")
                                nc.vector.tensor_copy(out=lg[:], in_=pL[:])
                                nc.sync.dma_start(
                                    p_logT[:, t * P:(t + 1) * P], lg[:])
                            if ag_after is not None:
                                ag_after(t)

            # annotate slot events with their open tile (builder convenience)
            def annotate(lp):
                ev2 = []
                cur = {}
                # map slot (s,j) -> tile via tile_slot_range
                s2t = {}
                for s in range(lp.nsec):
                    for t in range(lp.nt):
                        a, b = lp.tile_slot_range[s][t]
                        for j in range(a, b):
                            s2t[(s, j)] = t
                for e in lp.emit:
                    if e[0] == "slot":
                        ev2.append(("slot", e[1], e[2], s2t[(e[1], e[2])]))
                    else:
                        ev2.append(e)
                lp.emit = ev2
            annotate(pl.L1)
            annotate(pl.L2)

            def l1_ag(t):
                if "coll" in ab:
                    if t == NT1 - 1:
                        for k in range(NSEC2):
                            nc.sync.dma_start(
                                h1_full[k][0:pl.rows2[k], :],
                                h1_shard[pl.tb2[k] * P:pl.tb2[k + 1] * P, :])
                    return
                if AG_L2_AT_END:
                    if t == NT1 - 1:
                        for k in range(NSEC2):
                            allgather(
                                h1_shard[pl.tb2[k] * P:pl.tb2[k + 1] * P, :],
                                h1_full[k])
                    return
                bounds = list(pl.tb2[1:])
                if (t + 1) in bounds:
                    k = bounds.index(t + 1)
                    allgather(
                        h1_shard[pl.tb2[k] * P:pl.tb2[k + 1] * P, :],
                        h1_full[k])

            rgcn_layer(pl.L1, [h_full[s][:, :] for s in range(NSEC1)],
                       hall1, ww1, b1, 1, h1_shard, 1, ag_after=l1_ag)
            rgcn_layer(pl.L2, [h1_full[s][:, :] for s in range(NSEC2)],
                       hall2, ww2, b2, 2, None, 2)

    nc.compile()
    return nc


# ============================ host packing =============================

def pack_inputs(pl, inputs):
    f32 = np.float32
    sn = pl.sigma["shard_nodes"]  # [CORES, NSP], -1 pad
    vf = np.asarray(inputs["value_feature"], f32)
    tf = np.asarray(inputs["text_feature"], f32)

    def shard_textT(c):
        x = np.zeros((NSP, TEXT), f32)
        valid = sn[c] >= 0
        x[valid] = tf[sn[c][valid]]
        y = x.reshape(NT_MLP, P, TC, P).transpose(0, 3, 2, 1)
        return np.ascontiguousarray(y.reshape(NT_MLP, P, TC * P).astype(BF16))

    def shard_valT(c):
        x = np.zeros((NSP, VAL), f32)
        valid = sn[c] >= 0
        x[valid] = vf[sn[c][valid]]
        return np.ascontiguousarray(x.T.astype(BF16))

    fc1w = np.asarray(inputs["fc1_w"], f32)
    fc2w = np.asarray(inputs["fc2_w"], f32)
    relw = np.asarray(inputs["relu_w"], f32)
    beff = (np.concatenate([np.asarray(inputs["fc1_b"], f32),
                            np.asarray(inputs["fc2_b"], f32)]) @ relw
            + np.asarray(inputs["relu_b"], f32))
    fc2w_t = np.ascontiguousarray(
        fc2w.reshape(TC, P, FEAT).transpose(1, 0, 2)
        .reshape(P, TC * FEAT).astype(BF16))

    def stack_w(wrel, wroot):
        w = np.concatenate([np.asarray(wrel, f32),
                            np.asarray(wroot, f32)[None]], 0)
        return np.ascontiguousarray(
            w.transpose(1, 0, 2).reshape(P, RSLOT * FEAT).astype(BF16))

    # relation-major root block: node n -> column 3*P + n
    selfA = np.zeros((P, RSLOT * P), f32)
    selfA[np.arange(P), (RSLOT - 1) * P + np.arange(P)] = 1.0

    layout, blob_n = blob_layout(pl)
    shared = dict(
        fc1w=fc1w.astype(BF16), fc2w=fc2w_t,
        rwv=np.ascontiguousarray(relw[:FEAT].astype(BF16)),
        rwt=np.ascontiguousarray(relw[FEAT:].astype(BF16)),
        beff=beff[None].astype(BF16),
        ww1=stack_w(inputs["rgcn1_wrel"], inputs["rgcn1_wroot"]),
        b1=np.asarray(inputs["rgcn1_b"], f32)[None].astype(BF16),
        ww2=stack_w(inputs["rgcn2_wrel"], inputs["rgcn2_wroot"]),
        b2=np.asarray(inputs["rgcn2_b"], f32)[None].astype(BF16),
        fc3w=np.asarray(inputs["fc3_w"], f32).astype(BF16),
        fc3b=np.asarray(inputs["fc3_b"], f32)[None].astype(BF16),
        ones1=np.ones((1, P), f32).astype(BF16),
        selfA=selfA.astype(BF16),
    )

    def idxseg(arr):
        return (wrap16(arr.reshape(-1)) if arr.size
                else np.zeros((P, 8), np.int16)).view(BF16)

    in_maps = []
    for c in range(CORES):
        vals = dict(shared)
        vals["textT"] = shard_textT(c)
        vals["valT"] = shard_valT(c)
        vals["a1"] = pl.L1.amat[c] if pl.L1.nslot else np.zeros((P, W), BF16)
        vals["a2"] = pl.L2.amat[c] if pl.L2.nslot else np.zeros((P, W), BF16)
        vals["col1"] = pl.L1.cn[c][0].view(BF16)
        vals["nrm1"] = pl.L1.cn[c][1].view(BF16)
        vals["col2"] = pl.L2.cn[c][0].view(BF16)
        vals["nrm2"] = pl.L2.cn[c][1].view(BF16)
        for s in range(NSEC1):
            vals[f"idx1s{s}"] = idxseg(pl.L1.idx[c][s])
        for s in range(NSEC2):
            vals[f"idx2s{s}"] = idxseg(pl.L2.idx[c][s])
        blob = np.zeros((1, blob_n), BF16)
        for name, (off, n, shape) in layout.items():
            a = vals[name]
            assert a.size == n, (name, a.shape, shape)
            blob[0, off:off + n] = a.reshape(-1)
        in_maps.append({"blob": blob})
    return in_maps


# ============================ entry point =============================

_cache = {}


def kernel(**inputs):
    ei = np.asarray(inputs["edge_index"], np.int64)
    et = np.asarray(inputs["edge_type"], np.int64)
    idx = np.asarray(inputs["idx"], np.int64)

    key = hash((ei.tobytes(), et.tobytes(), idx.tobytes()))
    if key not in _cache:
        pl = make_plan(ei, et, idx)
        nc = build_bass(pl)
        _cache[key] = (pl, nc)
    pl, nc = _cache[key]

    in_maps = pack_inputs(pl, inputs)
    res = run_bass_kernel_spmd(nc, in_maps, list(range(CORES)))
    return assemble(pl, res, idx)


def assemble(pl, res, idx):
    sn = pl.sigma["shard_nodes"]
    logits = np.zeros((N_NODES, CLASSES), np.float32)
    for c in range(CORES):
        lt = res.results[c]["logitsT"]  # [2, NT2*P]
        nodes = sn[c][:pl.nt2 * P]
        vv = nodes >= 0
        logits[nodes[vv]] = lt[:, :len(nodes)][:, vv].T
    return logits[np.asarray(idx, np.int64)].astype(np.float32)


# revision 32
# speedup vs baseline: 1.5084x; 1.5084x over previous
"""BotRGCN Trainium2 kernel v3, 8-way SPMD.

Key changes vs v2:
- Relation-major scatter columns (ct = r*128 + dstrow): windows pack densely,
  fewer slots, contiguous per-relation epilogue slices.
- CHMAX=16 gather chunks (amortize ~1us SWDGE fixed overhead per call).
- h AllGather split into 4 section tables, issued inside the MLP loop.
- h1 AllGather split into 3 section tables, issued inside the L1 tile loop.
- Deferred last-section processing (D tiles) hides the final AllGather chunk.
- MLP processes 4 tiles per PSUM group (N=512) with text DMA on 4 engines.
- A matrices streamed in emission order (single sequential stream).
"""
import os
import sys

for _p in ("/opt/trn_rl_repo", "/root/.axon_site/_ro/trn_rl_repo"):
    if os.path.isdir(_p) and _p not in sys.path:
        sys.path.insert(0, _p)

import numpy as np
import ml_dtypes

from concourse import bass, bacc, tile, mybir
from concourse.bass_utils import run_bass_kernel_spmd

BF16 = ml_dtypes.bfloat16

N_NODES = 50000
N_REL = 3
FEAT = 128
VAL = 16
TEXT = 768
CLASSES = 2
CORES = 8
P = 128
W = 128
CHMAX = 8
ABATCH = 16
AGEN = False
RSLOT = 4
NSP = ((N_NODES // CORES) + P - 1) // P * P  # 6272
NT_MLP = NSP // P                            # 49
TC = TEXT // P                               # 6

# h table sections: MLP tile boundaries (per shard)
SB1 = [0, 17, 34, 49]
NSEC1 = len(SB1) - 1
# h1 table sections: L1 tile boundaries (set in make_plan from nt1)
NSEC2 = 2
D1 = 5   # L1 deferred tiles for last section
D2 = 3   # L2 deferred tiles for last section
AG_L2_AT_END = True  # issue all h1 AllGathers after L1 completes


def wrap16(flat):
    L = len(flat)
    assert L % 16 == 0
    a = np.asarray(flat, np.int16).reshape(-1, 16).T
    return np.ascontiguousarray(np.tile(a, (8, 1)))


# ============================ planner ================================


class Plan:
    pass


def _build_schedule(cts, cmax, w=W, cap=P):
    """Joint (cross-core) slot schedule for one (tile, section).
    cts: list of sorted int arrays (ct keys in [0, cmax)).
    Returns (bases, ranges)."""
    n = len(cts)
    ptrs = [0] * n
    lens = [len(a) for a in cts]
    bases, ranges = [], [[] for _ in range(n)]
    while any(ptrs[c] < lens[c] for c in range(n)):
        b = min(cts[c][ptrs[c]] for c in range(n) if ptrs[c] < lens[c])
        b = min(int(b), cmax - w)
        bases.append(b)
        for c in range(n):
            s = ptrs[c]
            hi = int(np.searchsorted(cts[c], b + w, side="left"))
            e = min(s + cap, max(hi, s))
            ranges[c].append((s, e))
            ptrs[c] = e
    return bases, ranges


def make_sigma(edge_index, edge_type, idx, n_nodes, cores, nsp):
    """Node permutation + pruning sets."""
    src = np.asarray(edge_index[0], np.int64)
    dst = np.asarray(edge_index[1], np.int64)

    idxset = np.unique(np.asarray(idx, np.int64))
    in_idx = np.zeros(n_nodes, bool)
    in_idx[idxset] = True

    m2 = in_idx[dst]                      # L2 edges
    l2src = np.unique(src[m2])
    needed = np.zeros(n_nodes, bool)
    needed[l2src] = True
    needed[idxset] = True
    m1 = needed[dst]                      # L1 edges

    D = idxset                             # idx dsts
    O = np.setdiff1d(np.flatnonzero(needed), D, assume_unique=False)
    U = np.flatnonzero(~needed)

    sd = [D[c::cores] for c in range(cores)]
    so = [O[c::cores] for c in range(cores)]
    su = [U[c::cores] for c in range(cores)]
    nd = max(len(x) for x in sd)
    nneed = max(len(sd[c]) + len(so[c]) for c in range(cores))
    nt2 = (nd + P - 1) // P
    nt1 = (nneed + P - 1) // P
    shard_nodes = np.full((cores, nsp), -1, np.int64)
    pos = np.full(n_nodes, -1, np.int64)
    for c in range(cores):
        arr = np.concatenate([sd[c], so[c], su[c]])
        assert len(arr) <= nsp, (len(arr), nsp)
        shard_nodes[c, :len(arr)] = arr
        pos[arr] = c * nsp + np.arange(len(arr))
    return dict(pos=pos, shard_nodes=shard_nodes, nt1=nt1, nt2=nt2,
                m1=m1, m2=m2, in_idx=in_idx, needed=needed,
                nd_per_core=np.array([len(x) for x in sd]))


def emission_events(lp, D):
    """Order in which the builder consumes slots.
    Yields ('open', t), ('slot', s, j), ('close', t).
    Sections 0..nsec-2 of tile t are emitted at t; the last section and the
    tile epilogue are deferred by D tiles."""
    ev = []
    nt, last = lp.nt, lp.nsec - 1
    for t in range(nt + D):
        if t < nt:
            ev.append(("open", t))
            for s in range(last):
                a, b = lp.tile_slot_range[s][t]
                ev += [("slot", s, j) for j in range(a, b)]
        if t >= D:
            td = t - D
            a, b = lp.tile_slot_range[last][td]
            ev += [("slot", last, j) for j in range(a, b)]
            ev.append(("close", td))
    return ev


def layer_plan(erow, ect, eowner, etile, enorm, cores, nt, sec, nsec, D):
    """Build joint slot schedule for one layer.

    erow: table-relative row per entry (int16 range); ect: ct key;
    eowner: dst core; etile: dst tile; enorm: edge norm; sec: table
    selector per entry in [0, nsec)."""
    SECS = tuple(range(nsec))
    order = np.lexsort((ect, sec, etile, eowner))
    erow, ect, sec = erow[order], ect[order], sec[order]
    eowner, etile, enorm = eowner[order], etile[order], enorm[order]

    key = (eowner * nt + etile) * nsec + sec
    bounds = np.searchsorted(key, np.arange(cores * nt * nsec + 1))

    slot_base = {s: [] for s in SECS}
    tile_slot_range = {s: np.zeros((nt, 2), np.int64) for s in SECS}
    idx16 = {s: [[] for _ in range(cores)] for s in SECS}
    acols = {s: [[] for _ in range(cores)] for s in SECS}
    anrm = {s: [[] for _ in range(cores)] for s in SECS}

    for t in range(nt):
        for s in SECS:
            cts_, rows_, nrms_ = [], [], []
            for c in range(cores):
                k = (c * nt + t) * nsec + s
                a, b = bounds[k], bounds[k + 1]
                cts_.append(ect[a:b])
                rows_.append(erow[a:b])
                nrms_.append(enorm[a:b])
            start = len(slot_base[s])
            bases, ranges = _build_schedule(cts_, P * RSLOT)
            for bj in bases:
                slot_base[s].append(bj)
            for c in range(cores):
                for j, (a, b) in enumerate(ranges[c]):
                    n = b - a
                    r = rows_[c][a:b]
                    cc = cts_[c][a:b] - bases[j]
                    nn = nrms_[c][a:b]
                    so_ = np.argsort(r, kind="stable")
                    r, cc, nn = r[so_], cc[so_], nn[so_]
                    ii = np.zeros(P, np.int16)
                    col = np.full(P, -1, np.int64)
                    nrm = np.zeros(P, np.float32)
                    assert n == 0 or r.max() < 32768
                    ii[:n] = r.astype(np.int16)
                    col[:n] = cc
                    nrm[:n] = nn
                    idx16[s][c].append(ii)
                    acols[s][c].append(col)
                    anrm[s][c].append(nrm)
            tile_slot_range[s][t] = (start, len(slot_base[s]))

    ns = [len(slot_base[s]) for s in SECS]
    nslot = sum(ns)
    out = Plan()
    out.nt, out.nslot, out.ns, out.nsec, out.D = nt, nslot, ns, nsec, D
    out.slot_base = {s: np.array(slot_base[s], np.int64) for s in SECS}
    out.tile_slot_range = tile_slot_range

    # per-core packed: idx streams per sec; A matrix in EMISSION order
    emit = emission_events(out, D)
    out.emit = emit
    slots_in_order = [(s, j) for (k, s, *r) in
                      [(e[0], e[1], *e[2:]) for e in emit] if False]
    slots_in_order = [(e[1], e[2]) for e in emit if e[0] == "slot"]
    out.slots_in_order = slots_in_order
    assert len(slots_in_order) == nslot
    out.idx = {}
    out.cn = {}
    out.amat = {}
    for c in range(cores):
        out.idx[c] = [
            (np.stack(idx16[s][c]) if idx16[s][c] else np.zeros((0, P), np.int16))
            for s in SECS
        ]
        colv = np.full((P, max(nslot, 1)), -1.0, np.float32)
        nrmv = np.zeros((P, max(nslot, 1)), np.float32)
        for g, (s, j) in enumerate(slots_in_order):
            colv[:, g] = acols[s][c][j].astype(np.float32)  # -1 for pads
            nrmv[:, g] = anrm[s][c][j]
        out.cn[c] = (colv, nrmv)
        am = np.zeros((P, max(nslot, 1) * W), BF16)
        for g, (s, j) in enumerate(slots_in_order):
            col = acols[s][c][j]
            nrm = anrm[s][c][j]
            v = col >= 0
            am[np.flatnonzero(v), g * W + col[v]] = nrm[v].astype(BF16)
        out.amat[c] = am
    return out


def make_plan(edge_index, edge_type, idx, n_nodes=50000, cores=8):
    src = np.asarray(edge_index[0], np.int64)
    dst = np.asarray(edge_index[1], np.int64)
    et = np.asarray(edge_type, np.int64)

    nsp = ((n_nodes // cores) + P - 1) // P * P  # 6272
    sg = make_sigma(edge_index, edge_type, idx, n_nodes, cores, nsp)
    pos, nt1, nt2 = sg["pos"], sg["nt1"], sg["nt2"]

    deg = np.zeros((N_REL, n_nodes), np.int64)
    np.add.at(deg, (et, dst), 1)
    norm = 1.0 / np.maximum(deg[et, dst], 1).astype(np.float32)

    pl = Plan()
    pl.cores, pl.nsp = cores, nsp
    pl.nt_mlp = nsp // P
    pl.sigma = sg
    pl.nt1, pl.nt2 = nt1, nt2

    # ---- L1: h split into NSEC1 section tables by MLP tile ranges
    rb1 = [b * P for b in SB1]              # per-shard row bases
    pl.rb1 = rb1
    rows1 = [rb1[s + 1] - rb1[s] for s in range(NSEC1)]
    pl.rows1 = rows1
    m1 = sg["m1"]
    s1, d1, r1, n1 = src[m1], dst[m1], et[m1], norm[m1]
    dpos = pos[d1]
    owner = dpos // nsp
    loc = dpos % nsp
    spos = pos[s1]
    sc, sl = spos // nsp, spos % nsp
    sec = np.searchsorted(np.array(rb1[1:]), sl, side="right")
    base = np.array([rb1[s] for s in range(NSEC1)])[sec]
    rows_s = np.array(rows1)[sec]
    erow = sc * rows_s + (sl - base)
    assert all(erow[sec == s].max(initial=0) < 32768 for s in range(NSEC1))
    ect = r1 * P + (loc % P)                # relation-major
    etile = loc // P
    assert (etile < nt1).all()
    pl.L1 = layer_plan(erow, ect, owner, etile, n1, cores, nt1, sec,
                       nsec=NSEC1, D=D1)

    # ---- L2: h1 split into NSEC2 section tables by L1 tile ranges
    tb2 = [0, min(32, nt1 - 1), nt1] if NSEC2 == 2 else \
        [min(k * ((nt1 + NSEC2 - 1) // NSEC2), nt1) for k in range(NSEC2 + 1)]
    pl.tb2 = tb2
    rb2 = [b * P for b in tb2]
    rows2 = [rb2[s + 1] - rb2[s] for s in range(NSEC2)]
    pl.rows2 = rows2
    m2 = sg["m2"]
    s2, d2, r2, n2 = src[m2], dst[m2], et[m2], norm[m2]
    dpos = pos[d2]
    owner = dpos // nsp
    loc = dpos % nsp
    assert (loc < nt2 * P).all()
    spos = pos[s2]
    sc, sl = spos // nsp, spos % nsp
    assert (sl < nt1 * P).all()
    sec = np.searchsorted(np.array(rb2[1:]), sl, side="right")
    base = np.array(rb2)[sec]
    rows_s = np.array(rows2)[sec]
    erow = sc * rows_s + (sl - base)
    assert all(erow[sec == s].max(initial=0) < 32768 for s in range(NSEC2))
    ect = r2 * P + (loc % P)
    etile = loc // P
    pl.L2 = layer_plan(erow, ect, owner, etile, n2, cores, nt2, sec,
                       nsec=NSEC2, D=D2)
    return pl


# ============================ blob layout =============================

def blob_layout(pl):
    n1, n2 = pl.L1.nslot, pl.L2.nslot
    segs = [
        ("textT", [NT_MLP, P, TC * P]),
        ("valT", [VAL, NSP]),
        ("fc1w", [VAL, FEAT]),
        ("fc2w", [P, TC * P]),
        ("rwv", [FEAT, FEAT]),
        ("rwt", [FEAT, FEAT]),
        ("beff", [1, FEAT]),
        ("ww1", [P, RSLOT * FEAT]),
        ("b1", [1, FEAT]),
        ("ww2", [P, RSLOT * FEAT]),
        ("b2", [1, FEAT]),
        ("fc3w", [FEAT, CLASSES]),
        ("fc3b", [1, CLASSES]),
        ("ones1", [1, P]),
        ("selfA", [P, RSLOT * P]),
        ("a1", [P, max(n1, 1) * W]),
        ("a2", [P, max(n2, 1) * W]),
        ("col1", [P, max(n1, 1) * 2]),
        ("nrm1", [P, max(n1, 1) * 2]),
        ("col2", [P, max(n2, 1) * 2]),
        ("nrm2", [P, max(n2, 1) * 2]),
    ] + [
        (f"idx1s{s}", [P, max(pl.L1.ns[s], 1) * 8]) for s in range(NSEC1)
    ] + [
        (f"idx2s{s}", [P, max(pl.L2.ns[s], 1) * 8]) for s in range(NSEC2)
    ]
    out = {}
    off = 0
    for name, shape in segs:
        n = int(np.prod(shape))
        out[name] = (off, n, shape)
        off += ((n + 127) // 128) * 128
    return out, off


# ============================ bass builder =============================

def build_bass(pl, ablate=()):
    ab = set(ablate)
    NT1, NT2 = pl.nt1, pl.nt2

    nc = bacc.Bacc("TRN2", target_bir_lowering=False, debug=False,
                   num_devices=CORES, num_swdge_queues=4,
                   dynamic_dma_scratch_size=49152)
    qrr = {"n": 0}
    dt = mybir.dt
    f32, bf, i16 = dt.float32, dt.bfloat16, dt.int16

    layout, blob_n = blob_layout(pl)
    p_blob = nc.declare_dram_parameter("blob", [1, blob_n], bf, isOutput=False)
    p_logT = nc.declare_dram_parameter("logitsT", [CLASSES, NT2 * P], f32,
                                       isOutput=True)

    def seg(name, dtype=bf):
        off, n, shape = layout[name]
        ap = p_blob[0:1, off:off + n]
        if dtype != bf:
            ap = ap.bitcast(dtype)
        r = int(np.prod(shape[:-1]))
        return ap.rearrange("o (r c) -> (o r) c", r=r)

    with tile.TileContext(nc) as tc:
        with tc.tile_pool(name="wt", bufs=1) as wt, \
             tc.tile_pool(name="sb", bufs=2) as sb, \
             tc.tile_pool(name="ep", bufs=12) as ep, \
             tc.tile_pool(name="ab1", bufs=6) as abp, \
             tc.tile_pool(name="tts", bufs=3) as tts, \
             tc.tile_pool(name="dram", bufs=1, space="DRAM") as dram:

            def resident(name, dtype=bf):
                off, n, shape = layout[name]
                t = wt.tile(list(shape[-2:] if len(shape) == 2 else shape),
                            dtype, tag=name)
                nc.sync.dma_start(t[:], seg(name, dtype))
                return t

            fc1w = resident("fc1w")
            valT = resident("valT")
            fc2w = resident("fc2w")
            rwv = resident("rwv")
            rwt = resident("rwt")
            beff = resident("beff")
            ones1 = resident("ones1")

            hall1 = wt.tile([P, NT1, P], bf, tag="hall1")
            hall2 = wt.tile([P, NT2, P], bf, tag="hall2")

            h_shard = dram.tile([NSP, FEAT], bf)
            _as = "Shared" if "coll" not in ab else "Local"
            h_full = [dram.tile([CORES * pl.rows1[s], FEAT], bf,
                                addr_space=_as, name=f"h_full{s}")
                      for s in range(NSEC1)]
            h1_shard = dram.tile([NT1 * P, FEAT], bf)
            h1_full = [dram.tile([CORES * pl.rows2[s], FEAT], bf,
                                 addr_space=_as, name=f"h1_full{s}")
                       for s in range(NSEC2)]
            warm_in = dram.tile([P, 16], bf)
            warm_out = dram.tile([CORES * P, 16], bf, addr_space=_as)

            def allgather(src_ap, dst):
                nc.gpsimd.collective_compute(
                    "AllGather", mybir.AluOpType.bypass,
                    replica_groups=[list(range(CORES))],
                    ins=[src_ap], outs=[dst.opt()])

            selfA = resident("selfA")
            iot16 = wt.tile([P, W], mybir.dt.int16, tag="iot16")
            nc.gpsimd.iota(iot16[:], pattern=[[1, W]], base=0,
                           channel_multiplier=0)
            iotb = wt.tile([P, W], bf, tag="iotb")
            nc.vector.tensor_copy(out=iotb[:], in_=iot16[:])

            def resident_f32(name, cols):
                t = wt.tile([P, cols], f32, tag=name)
                nc.sync.dma_start(t[:], seg(name, f32))
                return t

            cnr = {
                1: (resident_f32("col1", max(pl.L1.nslot, 1)),
                    resident_f32("nrm1", max(pl.L1.nslot, 1))),
                2: (resident_f32("col2", max(pl.L2.nslot, 1)),
                    resident_f32("nrm2", max(pl.L2.nslot, 1))),
            }
            ww1 = resident("ww1")
            b1 = resident("b1")
            ww2 = resident("ww2")
            b2 = resident("b2")
            fc3w = resident("fc3w")
            fc3b = resident("fc3b")
            idxsb = {
                1: [resident(f"idx1s{s}", i16) for s in range(NSEC1)],
                2: [resident(f"idx2s{s}", i16) for s in range(NSEC2)],
            }


            # ================= phase 1: feature MLP (quad tiles) ======
            text_engines = [nc.sync, nc.scalar, nc.sync]
            with tc.tile_pool(name="ps1", bufs=2, space="PSUM") as ps1:
                for t0 in range(0, NT_MLP, 4):
                    quad = min(4, NT_MLP - t0)
                    tt = tts.tile([P, TC, 4, P], bf, tag="tt")
                    for h_ in range(quad):
                        toff = layout["textT"][0] + (t0 + h_) * P * TC * P
                        text_engines[(t0 + h_) % 2].dma_start(
                            tt[:, :, h_, :],
                            p_blob[0:1, toff:toff + P * TC * P]
                            .rearrange("o (p c n) -> (o p) c n", p=P, c=TC))
                    np_ = quad * P
                    pvT = ps1.tile([P, 4, P], f32, tag="pvT", space="PSUM")
                    nc.tensor.matmul(out=pvT[:, 0:quad, :], lhsT=fc1w[:],
                                     rhs=valT[:, t0 * P:t0 * P + np_],
                                     start=True, stop=True)
                    vT = sb.tile([P, 4, P], bf, tag="vT")
                    nc.vector.tensor_copy(out=vT[:, 0:quad, :],
                                          in_=pvT[:, 0:quad, :])
                    ptT = ps1.tile([P, 4, P], f32, tag="ptT", space="PSUM")
                    for c in range(TC):
                        nc.tensor.matmul(out=ptT[:, 0:quad, :],
                                         lhsT=fc2w[:, c * P:(c + 1) * P],
                                         rhs=tt[:, c, 0:quad, :],
                                         start=(c == 0), stop=(c == TC - 1))
                    tT = sb.tile([P, 4, P], bf, tag="tT")
                    nc.vector.tensor_copy(out=tT[:, 0:quad, :],
                                          in_=ptT[:, 0:quad, :])
                    ph = ps1.tile([P, 4, P], f32, tag="ph", space="PSUM")
                    for h_ in range(quad):
                        nc.tensor.matmul(out=ph[:, h_, :], lhsT=vT[:, h_, :],
                                         rhs=rwv[:], start=True, stop=False)
                        nc.tensor.matmul(out=ph[:, h_, :], lhsT=tT[:, h_, :],
                                         rhs=rwt[:], start=False, stop=False)
                        nc.tensor.matmul(out=ph[:, h_, :], lhsT=ones1[:],
                                         rhs=beff[:], start=False, stop=True)
                    for h_ in range(quad):
                        t = t0 + h_
                        if t < NT1:
                            hdst = hall1[:, t, :]
                        else:
                            hsb = sb.tile([P, P], bf, tag="hsb")
                            hdst = hsb[:]
                        nc.scalar.activation(
                            out=hdst, in_=ph[:, h_, :],
                            func=mybir.ActivationFunctionType.Lrelu,
                            alpha=0.01)
                        nc.sync.dma_start(
                            h_shard[t * P:(t + 1) * P, :], hdst)
                        if (t + 1) in SB1[1:] and "coll" not in ab:
                            k = SB1.index(t + 1) - 1
                            allgather(
                                h_shard[pl.rb1[k]:pl.rb1[k + 1], :],
                                h_full[k])
            if "coll" in ab:
                for k in range(NSEC1):
                    nc.sync.dma_start(h_full[k][0:pl.rows1[k], :],
                                      h_shard[pl.rb1[k]:pl.rb1[k + 1], :])

            # ================= RGCN layers =================
            A_ENGINES = [nc.scalar, nc.sync]
            def rgcn_layer(lp, src_tabs, hall, ww, bb, layer, out_shards,
                           li, ag_after=None):
                nsec = lp.nsec
                emitted = {s: -1 for s in range(nsec)}
                ebufs = {s: {} for s in range(nsec)}
                nstream = {s: lp.ns[s] for s in range(nsec)}
                colr, nrmr = cnr[li]
                aemitted = [-1]
                abufs = {}
                aseg = seg("a1" if li == 1 else "a2")
                # global emission index per (s, j)
                gidx = {sj: g for g, sj in enumerate(lp.slots_in_order)}

                def emit_chunk(s, ci):
                    s0 = ci * CHMAX
                    ns = min(CHMAX, nstream[s] - s0)
                    et = ep.tile([P, CHMAX, FEAT], bf, tag=f"e{s}")
                    if "gather" in ab:
                        nc.vector.memset(et[:, 0:1, 0:2], 0.0)
                        ebufs[s][ci] = (et, s0, ns)
                        ebufs[s].pop(ci - 12, None)
                        return
                    qrr["n"] += 1
                    qn = qrr["n"] % 4
                    nc.gpsimd.dma_gather(
                        out_ap=et[:, 0:ns, :],
                        in_ap=src_tabs[s],
                        idxs_ap=idxsb[li][s][:, s0 * 8:(s0 + ns) * 8],
                        num_idxs=ns * P,
                        num_idxs_reg=ns * P,
                        elem_size=FEAT,
                        queue_num=qn)
                    ebufs[s][ci] = (et, s0, ns)
                    ebufs[s].pop(ci - 12, None)

                def gen_a(g):
                    at = abp.tile([P, W], bf, tag="ag", name="at")
                    nc.vector.tensor_scalar(
                        out=at[:], in0=iotb[:],
                        scalar1=colr[:, g:g + 1],
                        scalar2=nrmr[:, g:g + 1],
                        op0=mybir.AluOpType.is_equal,
                        op1=mybir.AluOpType.mult)
                    return at

                def emit_abatch(ai):
                    s0 = ai * ABATCH
                    ns = min(ABATCH, lp.nslot - s0)
                    at = abp.tile([P, ABATCH, W], bf, tag="a", name="abt")
                    A_ENGINES[ai % len(A_ENGINES)].dma_start(
                        at[:, 0:ns, :],
                        aseg[:, s0 * W:(s0 + ns) * W])
                    abufs[ai] = (at, s0, ns)
                    abufs.pop(ai - 4, None)

                with tc.tile_pool(name=f"psl{li}", bufs=lp.D + 1,
                                  space="PSUM") as psl, \
                     tc.tile_pool(name=f"pso{li}", bufs=2,
                                  space="PSUM") as pso:
                    pS = {}
                    for e in lp.emit:
                        if e[0] == "open":
                            t = e[1]
                            pS[t] = psl.tile([P, RSLOT * P], f32, tag="pS",
                                             name="pS", space="PSUM")
                            # root one-hot: also initializes all 512 cols
                            nc.tensor.matmul(out=pS[t][:], lhsT=hall[:, t, :],
                                             rhs=selfA[:], start=True,
                                             stop=False, skip_group_check=True)
                        elif e[0] == "slot":
                            s, j = e[1], e[2]
                            t = None  # tile known via pS accumulation target
                            ci = j // CHMAX
                            g = gidx[(s, j)]
                            if ci > emitted[s]:
                                emit_chunk(s, ci)
                                emitted[s] = ci
                            et, es0, _ = ebufs[s][ci]
                            if AGEN:
                                at = gen_a(g)
                                a_ap = at[:]
                            else:
                                ai = g // ABATCH
                                if ai > aemitted[0]:
                                    emit_abatch(ai)
                                    aemitted[0] = ai
                                at, as0, _ = abufs[ai]
                                a_ap = at[:, g - as0, :]
                            bj = int(lp.slot_base[s][j])
                            if "slotmm" in ab:
                                continue
                            tt_ = e[3]
                            nc.tensor.matmul(
                                out=pS[tt_][:, bj:bj + W],
                                lhsT=et[:, j - es0, :],
                                rhs=a_ap,
                                start=False, stop=False,
                                skip_group_check=True)
                        else:  # close
                            t = e[1]
                            sS = sb.tile([P, RSLOT * P], bf, tag="sS")
                            nc.scalar.activation(
                                out=sS[:], in_=pS[t][:],
                                func=mybir.ActivationFunctionType.Copy)
                            del pS[t]
                            if layer == 1:
                                pO = pso.tile([P, FEAT], f32, tag="pO",
                                              space="PSUM")
                                for r in range(RSLOT):
                                    nc.tensor.matmul(
                                        out=pO[:],
                                        lhsT=sS[:, r * P:(r + 1) * P],
                                        rhs=ww[:, r * FEAT:(r + 1) * FEAT],
                                        start=(r == 0), stop=False)
                                nc.tensor.matmul(out=pO[:], lhsT=ones1[:],
                                                 rhs=bb[:], start=False,
                                                 stop=True)
                                if t < NT2:
                                    hodst = hall2[:, t, :]
                                else:
                                    hot = sb.tile([P, P], bf, tag="ho")
                                    hodst = hot[:]
                                nc.vector.tensor_copy(out=hodst, in_=pO[:])
                                nc.sync.dma_start(
                                    out_shards[t * P:(t + 1) * P, :], hodst)
                            else:
                                pO = pso.tile([P, P], f32, tag="pO",
                                              space="PSUM")
                                for r in range(RSLOT):
                                    nc.tensor.matmul(
                                        out=pO[:],
                                        lhsT=ww[:, r * FEAT:(r + 1) * FEAT],
                                        rhs=sS[:, r * P:(r + 1) * P],
                                        start=(r == 0), stop=False)
                                nc.tensor.matmul(out=pO[:], lhsT=bb[:],
                                                 rhs=ones1[:], start=False,
                                                 stop=True)
                                h2T = sb.tile([P, P], bf, tag="h2T")
                                nc.vector.tensor_copy(out=h2T[:], in_=pO[:])
                                pL = pso.tile([CLASSES, P], f32, tag="pL",
                                              space="PSUM")
                                nc.tensor.matmul(out=pL[:], lhsT=fc3w[:],
                                                 rhs=h2T[:], start=True,
                                                 stop=False)
                                nc.tensor.matmul(out=pL[:], lhsT=fc3b[:],
                                                 rhs=ones1[:], start=False,
                                                 stop=True)
                                lg = sb.tile([CLASSES, P], f32, tag="lg")
                                nc.vector.tensor_copy(out=lg[:], in_=pL[:])
                                nc.sync.dma_start(
                                    p_logT[:, t * P:(t + 1) * P], lg[:])
                            if ag_after is not None:
                                ag_after(t)

            # annotate slot events with their open tile (builder convenience)
            def annotate(lp):
                ev2 = []
                cur = {}
                # map slot (s,j) -> tile via tile_slot_range
                s2t = {}
                for s in range(lp.nsec):
                    for t in range(lp.nt):
                        a, b = lp.tile_slot_range[s][t]
                        for j in range(a, b):
                            s2t[(s, j)] = t
                for e in lp.emit:
                    if e[0] == "slot":
                        ev2.append(("slot", e[1], e[2], s2t[(e[1], e[2])]))
                    else:
                        ev2.append(e)
                lp.emit = ev2
            annotate(pl.L1)
            annotate(pl.L2)

            def l1_ag(t):
                if "coll" in ab:
                    if t == NT1 - 1:
                        for k in range(NSEC2):
                            nc.sync.dma_start(
                                h1_full[k][0:pl.rows2[k], :],
                                h1_shard[pl.tb2[k] * P:pl.tb2[k + 1] * P, :])
                    return
                if AG_L2_AT_END:
                    if t == NT1 - 1:
                        for k in range(NSEC2):
                            allgather(
                                h1_shard[pl.tb2[k] * P:pl.tb2[k + 1] * P, :],
                                h1_full[k])
                    return
                bounds = list(pl.tb2[1:])
                if (t + 1) in bounds:
                    k = bounds.index(t + 1)
                    allgather(
                        h1_shard[pl.tb2[k] * P:pl.tb2[k + 1] * P, :],
                        h1_full[k])

            rgcn_layer(pl.L1, [h_full[s][:, :] for s in range(NSEC1)],
                       hall1, ww1, b1, 1, h1_shard, 1, ag_after=l1_ag)
            rgcn_layer(pl.L2, [h1_full[s][:, :] for s in range(NSEC2)],
                       hall2, ww2, b2, 2, None, 2)

    nc.compile()
    return nc


# ============================ host packing =============================

def pack_inputs(pl, inputs):
    f32 = np.float32
    sn = pl.sigma["shard_nodes"]  # [CORES, NSP], -1 pad
    vf = np.asarray(inputs["value_feature"], f32)
    tf = np.asarray(inputs["text_feature"], f32)

    def shard_textT(c):
        x = np.zeros((NSP, TEXT), f32)
        valid = sn[c] >= 0
        x[valid] = tf[sn[c][valid]]
        y = x.reshape(NT_MLP, P, TC, P).transpose(0, 3, 2, 1)
        return np.ascontiguousarray(y.reshape(NT_MLP, P, TC * P).astype(BF16))

    def shard_valT(c):
        x = np.zeros((NSP, VAL), f32)
        valid = sn[c] >= 0
        x[valid] = vf[sn[c][valid]]
        return np.ascontiguousarray(x.T.astype(BF16))

    fc1w = np.asarray(inputs["fc1_w"], f32)
    fc2w = np.asarray(inputs["fc2_w"], f32)
    relw = np.asarray(inputs["relu_w"], f32)
    beff = (np.concatenate([np.asarray(inputs["fc1_b"], f32),
                            np.asarray(inputs["fc2_b"], f32)]) @ relw
            + np.asarray(inputs["relu_b"], f32))
    fc2w_t = np.ascontiguousarray(
        fc2w.reshape(TC, P, FEAT).transpose(1, 0, 2)
        .reshape(P, TC * FEAT).astype(BF16))

    def stack_w(wrel, wroot):
        w = np.concatenate([np.asarray(wrel, f32),
                            np.asarray(wroot, f32)[None]], 0)
        return np.ascontiguousarray(
            w.transpose(1, 0, 2).reshape(P, RSLOT * FEAT).astype(BF16))

    # relation-major root block: node n -> column 3*P + n
    selfA = np.zeros((P, RSLOT * P), f32)
    selfA[np.arange(P), (RSLOT - 1) * P + np.arange(P)] = 1.0

    layout, blob_n = blob_layout(pl)
    shared = dict(
        fc1w=fc1w.astype(BF16), fc2w=fc2w_t,
        rwv=np.ascontiguousarray(relw[:FEAT].astype(BF16)),
        rwt=np.ascontiguousarray(relw[FEAT:].astype(BF16)),
        beff=beff[None].astype(BF16),
        ww1=stack_w(inputs["rgcn1_wrel"], inputs["rgcn1_wroot"]),
        b1=np.asarray(inputs["rgcn1_b"], f32)[None].astype(BF16),
        ww2=stack_w(inputs["rgcn2_wrel"], inputs["rgcn2_wroot"]),
        b2=np.asarray(inputs["rgcn2_b"], f32)[None].astype(BF16),
        fc3w=np.asarray(inputs["fc3_w"], f32).astype(BF16),
        fc3b=np.asarray(inputs["fc3_b"], f32)[None].astype(BF16),
        ones1=np.ones((1, P), f32).astype(BF16),
        selfA=selfA.astype(BF16),
    )

    def idxseg(arr):
        return (wrap16(arr.reshape(-1)) if arr.size
                else np.zeros((P, 8), np.int16)).view(BF16)

    in_maps = []
    for c in range(CORES):
        vals = dict(shared)
        vals["textT"] = shard_textT(c)
        vals["valT"] = shard_valT(c)
        vals["a1"] = pl.L1.amat[c] if pl.L1.nslot else np.zeros((P, W), BF16)
        vals["a2"] = pl.L2.amat[c] if pl.L2.nslot else np.zeros((P, W), BF16)
        vals["col1"] = pl.L1.cn[c][0].view(BF16)
        vals["nrm1"] = pl.L1.cn[c][1].view(BF16)
        vals["col2"] = pl.L2.cn[c][0].view(BF16)
        vals["nrm2"] = pl.L2.cn[c][1].view(BF16)
        for s in range(NSEC1):
            vals[f"idx1s{s}"] = idxseg(pl.L1.idx[c][s])
        for s in range(NSEC2):
            vals[f"idx2s{s}"] = idxseg(pl.L2.idx[c][s])
        blob = np.zeros((1, blob_n), BF16)
        for name, (off, n, shape) in layout.items():
            a = vals[name]
            assert a.size == n, (name, a.shape, shape)
            blob[0, off:off + n] = a.reshape(-1)
        in_maps.append({"blob": blob})
    return in_maps


# ============================ entry point =============================

_cache = {}


def kernel(**inputs):
    ei = np.asarray(inputs["edge_index"], np.int64)
    et = np.asarray(inputs["edge_type"], np.int64)
    idx = np.asarray(inputs["idx"], np.int64)

    key = hash((ei.tobytes(), et.tobytes(), idx.tobytes()))
    if key not in _cache:
        pl = make_plan(ei, et, idx)
        nc = build_bass(pl)
        _cache[key] = (pl, nc)
    pl, nc = _cache[key]

    in_maps = pack_inputs(pl, inputs)
    res = run_bass_kernel_spmd(nc, in_maps, list(range(CORES)))
    return assemble(pl, res, idx)


def assemble(pl, res, idx):
    sn = pl.sigma["shard_nodes"]
    logits = np.zeros((N_NODES, CLASSES), np.float32)
    for c in range(CORES):
        lt = res.results[c]["logitsT"]  # [2, NT2*P]
        nodes = sn[c][:pl.nt2 * P]
        vv = nodes >= 0
        logits[nodes[vv]] = lt[:, :len(nodes)][:, vv].T
    return logits[np.asarray(idx, np.int64)].astype(np.float32)
